# Initial kernel scaffold
#
"""Locally-connected conv (unshared weights) on 8 TRN2 NeuronCores.

Problem: inputs [64,32,32,64] f32, kernel [32,32,576,64] f32 (per-location
weights, KFEAT=3*3*64), bias [32,32,64] f32 -> out [64,32,32,64] f32
(SAME padding, stride 1).

Strategy (X-stationary, weight-streaming, bf16 compute):
  - Spatial shard: core c computes output rows 4c..4c+3 (host slices the
    zero-padded input with halo; no device collectives needed).
  - Weights are unshared -> each weight element is used exactly once, so
    they are the *moving* matmul operand, streamed from HBM in large
    contiguous chunks (DMA-bound problem: 151 MB of weights).
  - X patches are the *stationary* operand, reused across the 3x3
    neighborhood. K packs 2 input rows x 64 channels = 128.
  - Per output row pair (even i0, odd i1) and padded column c:
      M1: pair (i0,i0+1)   K=128 -> row i0 kh=(0,1) contributions
      M2: pair (i0+2,i0+3) K=128 -> row i1 kh=(1,2)
      M3: row i0+2 (K=64, parts 0:64)   -> row i0 kh=2
      M4: row i1   (K=64, parts 64:128) -> row i1 kh=0
    M3/M4 share stream columns (upper/lower partition halves).
  - PSUM: bank tile [128,512] holds both rows of a group: even row on
    partitions 0:64, odd row on 64:128 (via tile_position auto-derive
    from out base partition). All 4 output rows live in 8 banks.
  - Bias is added with one K=2 bf16 matmul per PSUM bank (lhsT = parity
    indicator matrix) which also initializes the bank (start=True).
  - Weight stream is chunk-major in HBM; fulls phase then a singles
    phase whose alternating K=64 row-groups let LDWEIGHTS pull ahead.
"""

import numpy as np
import ml_dtypes

import concourse.bass as bass  # noqa: F401
import concourse.mybir as mybir
import concourse.tile as tile
from concourse import bacc
from concourse.bass_utils import run_bass_kernel_spmd

BF16 = ml_dtypes.bfloat16

B, H, W, CIN, COUT = 64, 32, 32, 64, 64
KH, KW = 3, 3
KFEAT = KH * KW * CIN
NCORES = 8
RPC = H // NCORES              # output rows per core = 4
HP, WP = H + 2, W + 2          # zero-padded input dims
NPAIRS = 3                     # input row pairs per core (6 padded rows)
PAIR_COLS = WP * B             # 2176 free cols per pair tile
XP_COLS = NPAIRS * PAIR_COLS   # 6528
GROUPS = 2                     # output row pairs per core
BANKS = 4                      # psum banks per group
JPB = 8                        # output cols per bank (512 f32 / 64 co)
ROW_COLS = W * COUT            # 2048
# bias stream [2, BS_COLS]: cols 0:128 = parity indicator matrix
# (row0 = 1 on m<64, row1 = 1 on m>=64), then per-(g,bank) [2,512] bias
# blocks (row0 = even-row bias, row1 = odd-row bias).
BS_COLS = 128 + GROUPS * BANKS * 512  # 4224
MAX_CHUNK_COLS = 2304


def stream_layout():
    """Weight stream block order. Returns (records, chunks, total_cols).

    record = (g, c, typ, jset, col_off); typ 0=M1, 1=M2, 2=M34.
    chunks = list of (start_col, end_col), broken at (g,c) boundaries.
    """
    recs = []
    off = 0
    bounds = []
    for g in range(GROUPS):
        for phase_typs in ((0, 1), (2,)):
            for c in range(1, WP - 1):
                jset = [j for j in (c - 2, c - 1, c) if 0 <= j < W]
                if not jset:
                    continue
                for typ in phase_typs:
                    recs.append((g, c, typ, jset, off))
                    off += 64 * len(jset)
                bounds.append(off)
    chunks = []
    start, prev = 0, 0
    for b_ in bounds:
        cap = 1152 if not chunks else MAX_CHUNK_COLS
        if b_ - start > cap:
            chunks.append((start, prev))
            start = prev
        prev = b_
    chunks.append((start, prev))
    return recs, chunks, off


_RECS, _CHUNKS, TOTAL_COLS = stream_layout()


def mm_records():
    """Expand stream records into per-matmul records with psum targets."""
    chunk_of = {}
    for k, (a, b_) in enumerate(_CHUNKS):
        for g, c, typ, jset, off in _RECS:
            if a <= off < b_:
                chunk_of[off] = k
    mms = []
    for g, c, typ, jset, off in _RECS:
        # split jset (contiguous ascending) into per-bank pieces
        s = 0
        while s < len(jset):
            bk = jset[s] // JPB
            e = s
            while e < len(jset) and jset[e] // JPB == bk:
                e += 1
            c0 = off + s * 64
            c1 = off + e * 64
            o0 = (jset[s] % JPB) * 64
            o1 = o0 + (e - s) * 64
            if typ == 0:    # M1: row i0 (par 0), pair g, K=128
                sub = [(0, 128, g, 0)]
            elif typ == 1:  # M2: row i1 (par 1), pair g+1, K=128
                sub = [(0, 128, g + 1, 1)]
            else:           # M34: two K=64 matmuls sharing cols
                sub = [(0, 64, g + 1, 0), (64, 128, g, 1)]
            for (plo, phi, pair, par) in sub:
                mms.append(dict(g=g, bk=bk, par=par, plo=plo, phi=phi,
                                x0=pair * PAIR_COLS + c * 64,
                                c0=c0, c1=c1, o0=o0, o1=o1,
                                chunk=chunk_of[off]))
            s = e
    return mms


_weight_template_cache = [None]


def weight_template():
    """int64 [128, TOTAL_COLS]: flat index into core-0 kernel array."""
    if _weight_template_cache[0] is not None:
        return _weight_template_cache[0]
    T = np.empty((128, TOTAL_COLS), np.int64)
    co = np.arange(COUT)
    p = np.arange(128)
    ci = p % 64
    for g, c, typ, jset, off in _RECS:
        for jj, j in enumerate(jset):
            kw = c - j
            if typ == 0:
                i = np.full(128, 2 * g)
                kh = np.where(p < 64, 0, 1)
            elif typ == 1:
                i = np.full(128, 2 * g + 1)
                kh = np.where(p < 64, 1, 2)
            else:
                i = np.where(p < 64, 2 * g, 2 * g + 1)
                kh = np.where(p < 64, 2, 0)
            # conv_general_dilated_local flattens KFEAT as (ci, kh, kw)
            kf = ci * (KH * KW) + kh * KW + kw
            base = ((i * W + j) * KFEAT + kf) * COUT
            T[:, off + jj * 64: off + (jj + 1) * 64] = base[:, None] + co[None, :]
    _weight_template_cache[0] = T
    return T


def prep_in_maps(inputs, kernel, bias):
    inputs = np.asarray(inputs, np.float32)
    kernel = np.asarray(kernel, np.float32)
    bias = np.asarray(bias, np.float32)
    T = weight_template()
    kflat = np.ascontiguousarray(kernel).reshape(-1)
    xpad = np.zeros((B, HP, WP, CIN), np.float32)
    xpad[:, 1:H + 1, 1:W + 1, :] = inputs
    xpad = xpad.astype(BF16)
    in_maps = []
    for core in range(NCORES):
        rows = xpad[:, RPC * core: RPC * core + 6]          # [B, 6, WP, CIN]
        rt = rows.transpose(1, 3, 2, 0)                     # [r, ci, col, b]
        rt = rt.reshape(NPAIRS, 2, CIN, WP, B).transpose(1, 2, 0, 3, 4)
        xp = np.ascontiguousarray(rt.reshape(128, XP_COLS))  # [rip*ci, rp,col,b]
        woff = (RPC * core) * W * KFEAT * COUT
        wt = kflat[T + woff].astype(BF16)
        wt = np.concatenate([wt[:, a:b].reshape(-1) for a, b in _CHUNKS])
        bsh = bias[RPC * core: RPC * core + RPC].reshape(2 * GROUPS, ROW_COLS)
        bs = np.zeros((2, BS_COLS), np.float32)
        bs[0, 0:64] = 1.0
        bs[1, 64:128] = 1.0
        for g in range(GROUPS):
            for bk in range(BANKS):
                a = 128 + (g * BANKS + bk) * 512
                bs[0, a:a + 512] = bsh[2 * g, bk * 512:(bk + 1) * 512]
                bs[1, a:a + 512] = bsh[2 * g + 1, bk * 512:(bk + 1) * 512]
        in_maps.append({"xp": xp, "wt": wt, "bs": bs.astype(BF16)})
    return in_maps


def build_nc():
    dt = mybir.dt
    nc = bacc.Bacc(None, target_bir_lowering=False, debug=False)
    xp_d = nc.declare_dram_parameter("xp", [128, XP_COLS], dt.bfloat16,
                                     isOutput=False)
    wt_d = nc.declare_dram_parameter("wt", [128 * TOTAL_COLS], dt.bfloat16,
                                     isOutput=False)
    bs_d = nc.declare_dram_parameter("bs", [2, BS_COLS], dt.bfloat16,
                                     isOutput=False)
    out_d = nc.declare_dram_parameter("out", [GROUPS, BANKS, 128, 512],
                                      dt.bfloat16, isOutput=True)

    mms = mm_records()
    for m in mms:
        m["stop"] = False
    last_zr = {}
    last_bk = {}
    for idx, m in enumerate(mms):
        last_zr[(m["g"], m["bk"], m["par"])] = idx
        last_bk[(m["g"], m["bk"])] = idx
    for idx in last_zr.values():
        mms[idx]["stop"] = True
    evac_after = {idx: key for key, idx in last_bk.items()}

    with tile.TileContext(nc) as tc:
        with tc.tile_pool(name="const", bufs=1) as cpool, \
             tc.tile_pool(name="wpool", bufs=10) as wpool, \
             tc.tile_pool(name="opool", bufs=2) as opool, \
             tc.tile_pool(name="ps", bufs=1, space="PSUM") as pspool:
            bs_t = cpool.tile([2, BS_COLS], dt.bfloat16, name="bs_t",
                              tag="bs_t")
            nc.gpsimd.dma_start(out=bs_t[:], in_=bs_d[:])
            xp_t = cpool.tile([128, XP_COLS], dt.bfloat16, name="xp_t", tag="xp_t")
            nc.gpsimd.dma_start(out=xp_t[:, 0:2 * PAIR_COLS],
                                in_=xp_d[:, 0:2 * PAIR_COLS])
            ind = bs_t[0:2, 0:128]  # parity indicator matrix (lhsT)

            ps = {}
            for g in range(GROUPS):
                for bk in range(BANKS):
                    ps[(g, bk)] = pspool.tile([128, 512], dt.float32,
                                              name=f"ps{g}{bk}", tag=f"ps{g}{bk}")
            out_sb = {(g, bk): opool.tile([128, 512], dt.bfloat16,
                                          name=f"osb{g}{bk}", tag=f"osb{g}{bk}")
                      for g in range(GROUPS) for bk in range(BANKS)}

            # bias matmuls init psum (start=True): K=2 indicator trick
            # puts even-row bias on partitions 0:64, odd-row on 64:128
            # (fp32r matmuls cannot target dst partition 64+, so one
            # full-128-partition matmul per bank instead of two halves).
            for g in range(GROUPS):
                for bk in range(BANKS):
                    a = 128 + (g * BANKS + bk) * 512
                    rhs = bs_t[0:2, a:a + 512]
                    nc.tensor.matmul(ps[(g, bk)][0:128, :], ind, rhs,
                                     start=True, stop=False)

            cur_chunk = -1
            wtile = None
            for idx, m in enumerate(mms):
                if m["chunk"] != cur_chunk:
                    a, b_ = _CHUNKS[m["chunk"]]
                    wtile = wpool.tile([128, b_ - a], dt.bfloat16, name=f"wtile{m['chunk']}", tag="wt")
                    dma_eng = nc.sync if m["chunk"] % 2 == 0 else nc.scalar
                    dma_eng.dma_start(
                        out=wtile[:],
                        in_=wt_d[128 * a: 128 * b_].rearrange(
                            "(p n) -> p n", p=128))
                    if m["chunk"] == 2:
                        # pair 2 is first needed mid-kernel (group 1)
                        nc.gpsimd.dma_start(out=xp_t[:, 2 * PAIR_COLS:],
                                            in_=xp_d[:, 2 * PAIR_COLS:])
                    cur_chunk = m["chunk"]
                    coff = a
                lhsT = xp_t[m["plo"]:m["phi"], m["x0"]:m["x0"] + 64]
                rhs = wtile[m["plo"]:m["phi"], m["c0"] - coff:m["c1"] - coff]
                outap = ps[(m["g"], m["bk"])][
                    m["par"] * 64:(m["par"] + 1) * 64, m["o0"]:m["o1"]]
                nc.tensor.matmul(outap, lhsT, rhs, start=False, stop=m["stop"])
                if idx in evac_after:
                    g, bk = evac_after[idx]
                    nc.vector.tensor_copy(out=out_sb[(g, bk)][:],
                                          in_=ps[(g, bk)][:])
                    nc.scalar.dma_start(out=out_d[g, bk],
                                        in_=out_sb[(g, bk)][:])
    nc.compile()
    return nc


_NC_CACHE = [None]


def _get_nc():
    if _NC_CACHE[0] is None:
        _NC_CACHE[0] = build_nc()
    return _NC_CACHE[0]


def run_cores(in_maps, trace=False, **kw):
    nc = _get_nc()
    return run_bass_kernel_spmd(nc, in_maps, list(range(NCORES)),
                                trace=trace, **kw)


def unshard(results):
    y = np.empty((B, H, W, COUT), np.float32)
    for core in range(NCORES):
        o = np.asarray(results[core]["out"], np.float32)
        o = o.reshape(GROUPS, BANKS, 2, B, JPB, COUT)
        o = o.transpose(3, 0, 2, 1, 4, 5)  # [b, g, par, bk, j8, co]
        y[:, RPC * core: RPC * core + RPC] = o.reshape(B, RPC, W, COUT)
    return y


def kernel(inputs, kernel, bias):
    in_maps = prep_in_maps(inputs, kernel, bias)
    res = run_cores(in_maps)
    return unshard(res.results)



# revision 1
# speedup vs baseline: 1.0227x; 1.0227x over previous
"""Locally-connected conv (unshared weights) on 8 TRN2 NeuronCores.

Problem: inputs [64,32,32,64] f32, kernel [32,32,576,64] f32 (per-location
weights, KFEAT=3*3*64), bias [32,32,64] f32 -> out [64,32,32,64] f32
(SAME padding, stride 1).

Strategy (X-stationary, weight-streaming, bf16 compute):
  - Spatial shard: core c computes output rows 4c..4c+3 (host slices the
    zero-padded input with halo; no device collectives needed).
  - Weights are unshared -> each weight element is used exactly once, so
    they are the *moving* matmul operand, streamed from HBM in large
    contiguous chunks (DMA-bound problem: 151 MB of weights).
  - X patches are the *stationary* operand, reused across the 3x3
    neighborhood. K packs 2 input rows x 64 channels = 128.
  - Per output row pair (even i0, odd i1) and padded column c:
      M1: pair (i0,i0+1)   K=128 -> row i0 kh=(0,1) contributions
      M2: pair (i0+2,i0+3) K=128 -> row i1 kh=(1,2)
      M3: row i0+2 (K=64, parts 0:64)   -> row i0 kh=2
      M4: row i1   (K=64, parts 64:128) -> row i1 kh=0
    M3/M4 share stream columns (upper/lower partition halves).
  - PSUM: bank tile [128,512] holds both rows of a group: even row on
    partitions 0:64, odd row on 64:128 (via tile_position auto-derive
    from out base partition). All 4 output rows live in 8 banks.
  - Bias is added with one K=2 bf16 matmul per PSUM bank (lhsT = parity
    indicator matrix) which also initializes the bank (start=True).
  - Weight stream is chunk-major in HBM; fulls phase then a singles
    phase whose alternating K=64 row-groups let LDWEIGHTS pull ahead.
"""

import numpy as np
import ml_dtypes

import concourse.bass as bass  # noqa: F401
import concourse.mybir as mybir
import concourse.tile as tile
from concourse import bacc
from concourse.bass_utils import run_bass_kernel_spmd

BF16 = ml_dtypes.bfloat16

B, H, W, CIN, COUT = 64, 32, 32, 64, 64
KH, KW = 3, 3
KFEAT = KH * KW * CIN
NCORES = 8
RPC = H // NCORES              # output rows per core = 4
HP, WP = H + 2, W + 2          # zero-padded input dims
NPAIRS = 3                     # input row pairs per core (6 padded rows)
PAIR_COLS = WP * B             # 2176 free cols per pair tile
XP_COLS = NPAIRS * PAIR_COLS   # 6528
GROUPS = 2                     # output row pairs per core
BANKS = 4                      # psum banks per group
JPB = 8                        # output cols per bank (512 f32 / 64 co)
ROW_COLS = W * COUT            # 2048
# bias stream [2, BS_COLS]: cols 0:128 = parity indicator matrix
# (row0 = 1 on m<64, row1 = 1 on m>=64), then per-(g,bank) [2,512] bias
# blocks (row0 = even-row bias, row1 = odd-row bias).
BS_COLS = 128 + GROUPS * BANKS * 512  # 4224
MAX_CHUNK_COLS = 2304


def stream_layout():
    """Weight stream block order. Returns (records, chunks, total_cols).

    record = (g, c, typ, jset, col_off); typ 0=M1, 1=M2, 2=M34.
    chunks = list of (start_col, end_col), broken at (g,c) boundaries.
    """
    recs = []
    off = 0
    bounds = []
    for g in range(GROUPS):
        for phase_typs in ((0, 1), (2,)):
            for c in range(1, WP - 1):
                jset = [j for j in (c - 2, c - 1, c) if 0 <= j < W]
                if not jset:
                    continue
                for typ in phase_typs:
                    recs.append((g, c, typ, jset, off))
                    off += 64 * len(jset)
                bounds.append(off)
    chunks = []
    start, prev = 0, 0
    for b_ in bounds:
        cap = 1152 if not chunks else MAX_CHUNK_COLS
        if b_ - start > cap:
            chunks.append((start, prev))
            start = prev
        prev = b_
    chunks.append((start, prev))
    return recs, chunks, off


_RECS, _CHUNKS, TOTAL_COLS = stream_layout()


def mm_records():
    """Expand stream records into per-matmul records with psum targets."""
    chunk_of = {}
    for k, (a, b_) in enumerate(_CHUNKS):
        for g, c, typ, jset, off in _RECS:
            if a <= off < b_:
                chunk_of[off] = k
    mms = []
    for g, c, typ, jset, off in _RECS:
        # split jset (contiguous ascending) into per-bank pieces
        s = 0
        while s < len(jset):
            bk = jset[s] // JPB
            e = s
            while e < len(jset) and jset[e] // JPB == bk:
                e += 1
            c0 = off + s * 64
            c1 = off + e * 64
            o0 = (jset[s] % JPB) * 64
            o1 = o0 + (e - s) * 64
            if typ == 0:    # M1: row i0 (par 0), pair g, K=128
                sub = [(0, 128, g, 0)]
            elif typ == 1:  # M2: row i1 (par 1), pair g+1, K=128
                sub = [(0, 128, g + 1, 1)]
            else:           # M34: two K=64 matmuls sharing cols
                sub = [(0, 64, g + 1, 0), (64, 128, g, 1)]
            for (plo, phi, pair, par) in sub:
                mms.append(dict(g=g, bk=bk, par=par, plo=plo, phi=phi,
                                x0=pair * PAIR_COLS + c * 64,
                                c0=c0, c1=c1, o0=o0, o1=o1,
                                chunk=chunk_of[off]))
            s = e
    return mms


_weight_template_cache = [None]


def weight_template():
    """int64 [128, TOTAL_COLS]: flat index into core-0 kernel array."""
    if _weight_template_cache[0] is not None:
        return _weight_template_cache[0]
    T = np.empty((128, TOTAL_COLS), np.int64)
    co = np.arange(COUT)
    p = np.arange(128)
    ci = p % 64
    for g, c, typ, jset, off in _RECS:
        for jj, j in enumerate(jset):
            kw = c - j
            if typ == 0:
                i = np.full(128, 2 * g)
                kh = np.where(p < 64, 0, 1)
            elif typ == 1:
                i = np.full(128, 2 * g + 1)
                kh = np.where(p < 64, 1, 2)
            else:
                i = np.where(p < 64, 2 * g, 2 * g + 1)
                kh = np.where(p < 64, 2, 0)
            # conv_general_dilated_local flattens KFEAT as (ci, kh, kw)
            kf = ci * (KH * KW) + kh * KW + kw
            base = ((i * W + j) * KFEAT + kf) * COUT
            T[:, off + jj * 64: off + (jj + 1) * 64] = base[:, None] + co[None, :]
    _weight_template_cache[0] = T
    return T


def prep_in_maps(inputs, kernel, bias):
    inputs = np.asarray(inputs, np.float32)
    kernel = np.asarray(kernel, np.float32)
    bias = np.asarray(bias, np.float32)
    T = weight_template()
    kflat = np.ascontiguousarray(kernel).reshape(-1)
    xpad = np.zeros((B, HP, WP, CIN), np.float32)
    xpad[:, 1:H + 1, 1:W + 1, :] = inputs
    xpad = xpad.astype(BF16)
    in_maps = []
    for core in range(NCORES):
        rows = xpad[:, RPC * core: RPC * core + 6]          # [B, 6, WP, CIN]
        rt = rows.transpose(1, 3, 2, 0)                     # [r, ci, col, b]
        rt = rt.reshape(NPAIRS, 2, CIN, WP, B).transpose(1, 2, 0, 3, 4)
        xp = np.ascontiguousarray(rt.reshape(128, XP_COLS))  # [rip*ci, rp,col,b]
        woff = (RPC * core) * W * KFEAT * COUT
        wt = kflat[T + woff].astype(BF16)
        wt = np.concatenate([wt[:, a:b].reshape(-1) for a, b in _CHUNKS])
        bsh = bias[RPC * core: RPC * core + RPC].reshape(2 * GROUPS, ROW_COLS)
        bs = np.zeros((2, BS_COLS), np.float32)
        bs[0, 0:64] = 1.0
        bs[1, 64:128] = 1.0
        for g in range(GROUPS):
            for bk in range(BANKS):
                a = 128 + (g * BANKS + bk) * 512
                bs[0, a:a + 512] = bsh[2 * g, bk * 512:(bk + 1) * 512]
                bs[1, a:a + 512] = bsh[2 * g + 1, bk * 512:(bk + 1) * 512]
        in_maps.append({"xp": xp, "wt": wt, "bs": bs.astype(BF16)})
    return in_maps


def build_nc():
    dt = mybir.dt
    nc = bacc.Bacc(None, target_bir_lowering=False, debug=False)
    xp_d = nc.declare_dram_parameter("xp", [128, XP_COLS], dt.bfloat16,
                                     isOutput=False)
    wt_d = nc.declare_dram_parameter("wt", [128 * TOTAL_COLS], dt.bfloat16,
                                     isOutput=False)
    bs_d = nc.declare_dram_parameter("bs", [2, BS_COLS], dt.bfloat16,
                                     isOutput=False)
    out_d = nc.declare_dram_parameter("out", [GROUPS, BANKS, 128, 512],
                                      dt.bfloat16, isOutput=True)

    mms = mm_records()
    for m in mms:
        m["stop"] = False
    last_zr = {}
    last_bk = {}
    for idx, m in enumerate(mms):
        last_zr[(m["g"], m["bk"], m["par"])] = idx
        last_bk[(m["g"], m["bk"])] = idx
    for idx in last_zr.values():
        mms[idx]["stop"] = True
    evac_after = {idx: key for key, idx in last_bk.items()}

    with tile.TileContext(nc) as tc:
        with tc.tile_pool(name="const", bufs=1) as cpool, \
             tc.tile_pool(name="wpool", bufs=10) as wpool, \
             tc.tile_pool(name="opool", bufs=2) as opool, \
             tc.tile_pool(name="ps", bufs=1, space="PSUM") as pspool:
            bs_t = cpool.tile([2, BS_COLS], dt.bfloat16, name="bs_t",
                              tag="bs_t")
            nc.gpsimd.dma_start(out=bs_t[:], in_=bs_d[:])
            xp_t = cpool.tile([128, XP_COLS], dt.bfloat16, name="xp_t", tag="xp_t")
            nc.gpsimd.dma_start(out=xp_t[:, 0:2 * PAIR_COLS],
                                in_=xp_d[:, 0:2 * PAIR_COLS])
            ind = bs_t[0:2, 0:128]  # parity indicator matrix (lhsT)

            ps = {}
            for g in range(GROUPS):
                for bk in range(BANKS):
                    ps[(g, bk)] = pspool.tile([128, 512], dt.float32,
                                              name=f"ps{g}{bk}", tag=f"ps{g}{bk}")
            out_sb = {(g, bk): opool.tile([128, 512], dt.bfloat16,
                                          name=f"osb{g}{bk}", tag=f"osb{g}{bk}")
                      for g in range(GROUPS) for bk in range(BANKS)}

            # bias matmuls init psum (start=True): K=2 indicator trick
            # puts even-row bias on partitions 0:64, odd-row on 64:128
            # (fp32r matmuls cannot target dst partition 64+, so one
            # full-128-partition matmul per bank instead of two halves).
            for g in range(GROUPS):
                for bk in range(BANKS):
                    a = 128 + (g * BANKS + bk) * 512
                    rhs = bs_t[0:2, a:a + 512]
                    nc.tensor.matmul(ps[(g, bk)][0:128, :], ind, rhs,
                                     start=True, stop=False)

            cur_chunk = -1
            wtile = None
            for idx, m in enumerate(mms):
                if m["chunk"] != cur_chunk:
                    a, b_ = _CHUNKS[m["chunk"]]
                    wtile = wpool.tile([128, b_ - a], dt.bfloat16, name=f"wtile{m['chunk']}", tag="wt")
                    dma_eng = nc.sync if m["chunk"] % 2 == 0 else nc.scalar
                    dma_eng.dma_start(
                        out=wtile[:],
                        in_=wt_d[128 * a: 128 * b_].rearrange(
                            "(p n) -> p n", p=128))
                    if m["chunk"] == 2:
                        # pair 2 is first needed mid-kernel (group 1)
                        nc.gpsimd.dma_start(out=xp_t[:, 2 * PAIR_COLS:],
                                            in_=xp_d[:, 2 * PAIR_COLS:])
                    cur_chunk = m["chunk"]
                    coff = a
                lhsT = xp_t[m["plo"]:m["phi"], m["x0"]:m["x0"] + 64]
                rhs = wtile[m["plo"]:m["phi"], m["c0"] - coff:m["c1"] - coff]
                outap = ps[(m["g"], m["bk"])][
                    m["par"] * 64:(m["par"] + 1) * 64, m["o0"]:m["o1"]]
                nc.tensor.matmul(outap, lhsT, rhs, start=False, stop=m["stop"])
                if idx in evac_after:
                    g, bk = evac_after[idx]
                    nc.vector.tensor_copy(out=out_sb[(g, bk)][:],
                                          in_=ps[(g, bk)][:])
                    nc.scalar.dma_start(out=out_d[g, bk],
                                        in_=out_sb[(g, bk)][:])
    nc.compile()
    return nc


_NC_CACHE = [None]


def _get_nc():
    if _NC_CACHE[0] is None:
        _NC_CACHE[0] = build_nc()
    return _NC_CACHE[0]


def run_cores(in_maps, trace=False, **kw):
    nc = _get_nc()
    return run_bass_kernel_spmd(nc, in_maps, list(range(NCORES)),
                                trace=trace, **kw)


def unshard(results):
    y = np.empty((B, H, W, COUT), np.float32)
    for core in range(NCORES):
        o = np.asarray(results[core]["out"], np.float32)
        o = o.reshape(GROUPS, BANKS, 2, B, JPB, COUT)
        o = o.transpose(3, 0, 2, 1, 4, 5)  # [b, g, par, bk, j8, co]
        y[:, RPC * core: RPC * core + RPC] = o.reshape(B, RPC, W, COUT)
    return y


def kernel(inputs, kernel, bias):
    in_maps = prep_in_maps(inputs, kernel, bias)
    res = run_cores(in_maps)
    return unshard(res.results)

